# revision 7
# baseline (speedup 1.0000x reference)
"""Trainium2 Bass kernel for nn_DigitConvolutionalModel (3x3 conv + 3-layer MLP).

Math: out = relu(relu(conv3x3(x) @ W1 + b1) @ W2 + b2) @ W3 + b3.

The 3x3 valid conv is linear, so on host we fold it into the first FC:
  conv_flat = x @ A  with A [784, 676] (9 shifted diagonals of conv_w), so
  h1 = relu(x @ W1eff + b1)  with  W1eff = A @ W1 : [784, 256].

Sharding: pure data parallel over the batch across 8 cores (2048 rows each).
Each core runs a feature-major 3-layer MLP (activations stored transposed so
every matmul uses the weights as stored, with zero on-device transposes):
  h1T = relu(W1eff.T @ xT + b1)   [256, 2048]
  h2T = relu(W2.T   @ h1T + b2)   [256, 2048]
  oT  =      W3.T   @ h2T + b3    [10, 2048]
The host transposes each x shard on the way in and the outputs on the way out.
"""

import numpy as np

import concourse.bacc as bacc
import concourse.bass as bass
import concourse.mybir as mybir
import concourse.tile as tile
from concourse.bass_utils import run_bass_kernel_spmd

N_CORES = 8
B = 16384
B_LOC = B // N_CORES  # 2048 batch rows per core
NCH = 512  # batch chunk per matmul (fp32 PSUM bank = 512 floats)
KIN = 784  # folded input features (28*28)
H = 256
NOUT = 10

F32 = mybir.dt.float32
F32R = mybir.dt.float32r
AF = mybir.ActivationFunctionType
ALU = mybir.AluOpType

# K-tiles of the 784-long contraction: six of 128 plus a 16 tail
K1_TILES = [(k, min(128, KIN - k)) for k in range(0, KIN, 128)]


def build_nc() -> bass.Bass:
    nc = bacc.Bacc(
        "TRN2", target_bir_lowering=False, debug=False, num_devices=N_CORES
    )
    xT = nc.dram_tensor("xT", [KIN, B_LOC], F32R, kind="ExternalInput")
    w1 = nc.dram_tensor("w1", [KIN, H], F32R, kind="ExternalInput")
    b1 = nc.dram_tensor("b1", [H, 1], F32, kind="ExternalInput")
    w2 = nc.dram_tensor("w2", [H, H], F32R, kind="ExternalInput")
    b2 = nc.dram_tensor("b2", [H, 1], F32, kind="ExternalInput")
    w3 = nc.dram_tensor("w3", [H, NOUT], F32R, kind="ExternalInput")
    b3 = nc.dram_tensor("b3", [NOUT, 1], F32, kind="ExternalInput")
    outT = nc.dram_tensor("outT", [NOUT, B_LOC], F32, kind="ExternalOutput")

    nk1 = len(K1_TILES)

    with tile.TileContext(nc) as tc:
        with (
            tc.tile_pool(name="wgt", bufs=1) as wp,
            tc.tile_pool(name="xin", bufs=3) as xp,
            tc.tile_pool(name="act", bufs=2) as hp,
            tc.tile_pool(name="osb", bufs=2) as op,
            tc.tile_pool(name="ps1", bufs=2, space="PSUM") as pp1,
            tc.tile_pool(name="ps2", bufs=1, space="PSUM") as pp2,
            tc.tile_pool(name="ps3", bufs=2, space="PSUM") as pp3,
        ):
            # ---- resident weights/biases -------------------------------
            w1t = []
            for k0, kp in K1_TILES:
                t = wp.tile([kp, H], F32R, name=f"w1_{k0}")
                nc.sync.dma_start(out=t[:], in_=w1[k0 : k0 + kp, :])
                w1t.append(t)
            w2t = []
            for k in range(2):
                t = wp.tile([128, H], F32R, name=f"w2_{k}")
                nc.sync.dma_start(out=t[:], in_=w2[k * 128 : (k + 1) * 128, :])
                w2t.append(t)
            w3t = []
            for k in range(2):
                t = wp.tile([128, NOUT], F32R, name=f"w3_{k}")
                nc.sync.dma_start(out=t[:], in_=w3[k * 128 : (k + 1) * 128, :])
                w3t.append(t)
            # Biases are staged through a same-engine copy so the consuming
            # ACT/DVE instruction depends on its own engine (program order)
            # instead of a DMA semaphore — walrus allows only one sync-wait
            # slot on Activation instructions.
            b1t = []
            b2t = []
            for m in range(2):
                s = wp.tile([128, 1], F32, name=f"b1s_{m}")
                nc.sync.dma_start(out=s[:], in_=b1[m * 128 : (m + 1) * 128, :])
                t = wp.tile([128, 1], F32, name=f"b1_{m}")
                nc.vector.tensor_copy(t[:], s[:])
                b1t.append(t)
                s = wp.tile([128, 1], F32, name=f"b2s_{m}")
                nc.sync.dma_start(out=s[:], in_=b2[m * 128 : (m + 1) * 128, :])
                t = wp.tile([128, 1], F32, name=f"b2_{m}")
                nc.vector.tensor_copy(t[:], s[:])
                b2t.append(t)
            b3s = wp.tile([NOUT, 1], F32, name="b3s")
            nc.sync.dma_start(out=b3s[:], in_=b3[:, :])
            b3t = wp.tile([NOUT, 1], F32, name="b3")
            nc.vector.tensor_copy(b3t[:], b3s[:])

            # ---- batch-chunk pipeline ----------------------------------
            for ci, n0 in enumerate(range(0, B_LOC, NCH)):
                xts = []
                for k0, kp in K1_TILES:
                    t = xp.tile([kp, NCH], F32R, name="xt", tag=f"x_{k0}")
                    nc.sync.dma_start(out=t[:], in_=xT[k0 : k0 + kp, n0 : n0 + NCH])
                    xts.append(t)

                # layer 1: h1T = relu(W1eff.T @ xT + b1)
                h1 = []
                for m in range(2):
                    ps = pp1.tile([128, NCH], F32, name="ps1", tag=f"ps1_{m}")
                    for ki in range(nk1):
                        nc.tensor.matmul(
                            ps[:],
                            w1t[ki][:, m * 128 : (m + 1) * 128],
                            xts[ki][:],
                            start=(ki == 0),
                            stop=(ki == nk1 - 1),
                        )
                    h = hp.tile([128, NCH], F32R, name="h1", tag=f"h1_{m}")
                    nc.vector.tensor_scalar(
                        h[:], ps[:], b1t[m][:], 0.0, ALU.add, ALU.max
                    )
                    h1.append(h)

                # layer 2: h2T = relu(W2.T @ h1T + b2)
                h2 = []
                for m in range(2):
                    ps = pp2.tile([128, NCH], F32, name="ps2", tag=f"ps2_{m}")
                    for k in range(2):
                        nc.tensor.matmul(
                            ps[:],
                            w2t[k][:, m * 128 : (m + 1) * 128],
                            h1[k][:],
                            start=(k == 0),
                            stop=(k == 1),
                        )
                    h = hp.tile([128, NCH], F32R, name="h2", tag=f"h2_{m}")
                    nc.vector.tensor_scalar(
                        h[:], ps[:], b2t[m][:], 0.0, ALU.add, ALU.max
                    )
                    h2.append(h)

                # layer 3: oT = W3.T @ h2T + b3
                ps = pp3.tile([NOUT, NCH], F32, name="ps3", tag="ps3")
                for k in range(2):
                    nc.tensor.matmul(
                        ps[:],
                        w3t[k][:],
                        h2[k][:],
                        start=(k == 0),
                        stop=(k == 1),
                    )
                ob = op.tile([NOUT, NCH], F32, name="ob", tag="ob")
                nc.vector.tensor_scalar(ob[:], ps[:], b3t[:], None, ALU.add)
                nc.sync.dma_start(out=outT[:, n0 : n0 + NCH], in_=ob[:])

    nc.compile()
    return nc


def _fold_conv_into_w1(conv_w: np.ndarray, W1: np.ndarray) -> np.ndarray:
    """W1eff[784, 256] such that x @ W1eff == conv_flat(x, conv_w) @ W1."""
    W1v = W1.astype(np.float64).reshape(26, 26, W1.shape[1])
    cw = conv_w.astype(np.float64)
    acc = np.zeros((28, 28, W1.shape[1]), np.float64)
    for di in range(3):
        for dj in range(3):
            acc[di : di + 26, dj : dj + 26, :] += cw[di, dj] * W1v
    return acc.reshape(KIN, W1.shape[1]).astype(np.float32)


def _run(inputs: dict, trace: bool = False, tmpdir: str | None = None):
    x = np.ascontiguousarray(np.asarray(inputs["x"], dtype=np.float32))
    w1e = _fold_conv_into_w1(np.asarray(inputs["conv_w"]), np.asarray(inputs["W1"]))
    b1 = np.ascontiguousarray(np.asarray(inputs["b1"], np.float32).reshape(H, 1))
    w2 = np.ascontiguousarray(np.asarray(inputs["W2"], np.float32))
    b2 = np.ascontiguousarray(np.asarray(inputs["b2"], np.float32).reshape(H, 1))
    w3 = np.ascontiguousarray(np.asarray(inputs["W3"], np.float32))
    b3 = np.ascontiguousarray(np.asarray(inputs["b3"], np.float32).reshape(NOUT, 1))

    nc = build_nc()
    in_maps = []
    for c in range(N_CORES):
        xs = np.ascontiguousarray(x[c * B_LOC : (c + 1) * B_LOC].T)
        in_maps.append(
            {"xT": xs, "w1": w1e, "b1": b1, "w2": w2, "b2": b2, "w3": w3, "b3": b3}
        )

    res = run_bass_kernel_spmd(
        nc, in_maps, list(range(N_CORES)), trace=trace, tmpdir=tmpdir
    )
    out = np.concatenate([r["outT"].T for r in res.results], axis=0)
    return np.ascontiguousarray(out.astype(np.float32)), res


def kernel(**inputs) -> np.ndarray:
    out, _ = _run(inputs, trace=False)
    return out


# revision 8
# speedup vs baseline: 1.0091x; 1.0091x over previous
"""Trainium2 Bass kernel for nn_DigitConvolutionalModel (3x3 conv + 3-layer MLP).

Math: out = relu(relu(conv3x3(x) @ W1 + b1) @ W2 + b2) @ W3 + b3.

The 3x3 valid conv is linear, so on host we fold it into the first FC:
  conv_flat = x @ A  with A [784, 676] (9 shifted diagonals of conv_w), so
  h1 = relu(x @ W1eff + b1)  with  W1eff = A @ W1 : [784, 256].

Sharding: pure data parallel over the batch across 8 cores (2048 rows each).
Each core runs a feature-major 3-layer MLP (activations stored transposed so
every matmul uses the weights as stored, with zero on-device transposes):
  h1T = relu(W1eff.T @ xT + b1)   [256, 2048]
  h2T = relu(W2.T   @ h1T + b2)   [256, 2048]
  oT  =      W3.T   @ h2T + b3    [10, 2048]
The host transposes each x shard on the way in and the outputs on the way out.
"""

import numpy as np

import concourse.bacc as bacc
import concourse.bass as bass
import concourse.mybir as mybir
import concourse.tile as tile
from concourse.bass_utils import run_bass_kernel_spmd

N_CORES = 8
B = 16384
B_LOC = B // N_CORES  # 2048 batch rows per core
NCH = 512  # batch chunk per matmul (fp32 PSUM bank = 512 floats)
KIN = 784  # folded input features (28*28)
H = 256
NOUT = 10

F32 = mybir.dt.float32
F32R = mybir.dt.float32r
AF = mybir.ActivationFunctionType
ALU = mybir.AluOpType

# K-tiles of the 784-long contraction: six of 128 plus a 16 tail
K1_TILES = [(k, min(128, KIN - k)) for k in range(0, KIN, 128)]


def build_nc() -> bass.Bass:
    nc = bacc.Bacc(
        "TRN2", target_bir_lowering=False, debug=False, num_devices=N_CORES
    )
    xT = nc.dram_tensor("xT", [KIN, B_LOC], F32R, kind="ExternalInput")
    w1 = nc.dram_tensor("w1", [KIN, H], F32R, kind="ExternalInput")
    b1 = nc.dram_tensor("b1", [H, 1], F32, kind="ExternalInput")
    w2 = nc.dram_tensor("w2", [H, H], F32R, kind="ExternalInput")
    b2 = nc.dram_tensor("b2", [H, 1], F32, kind="ExternalInput")
    w3 = nc.dram_tensor("w3", [H, NOUT], F32R, kind="ExternalInput")
    b3 = nc.dram_tensor("b3", [NOUT, 1], F32, kind="ExternalInput")
    outT = nc.dram_tensor("outT", [NOUT, B_LOC], F32, kind="ExternalOutput")

    nk1 = len(K1_TILES)

    with tile.TileContext(nc) as tc:
        with (
            tc.tile_pool(name="wgt", bufs=1) as wp,
            tc.tile_pool(name="xin", bufs=2) as xp,
            tc.tile_pool(name="act", bufs=2) as hp,
            tc.tile_pool(name="osb", bufs=2) as op,
            tc.tile_pool(name="ps1", bufs=2, space="PSUM") as pp1,
            tc.tile_pool(name="ps2", bufs=1, space="PSUM") as pp2,
            tc.tile_pool(name="ps3", bufs=2, space="PSUM") as pp3,
        ):
            # DMA issue order is consumption order (the SDMA engines
            # round-robin across everything in flight, so whatever is queued
            # first finishes first — front-load only what chunk 0 needs):
            # W1[m=0] slices + chunk-0 x, then W1[m=1], then the small stuff.
            def load_x_chunk(n0):
                xts = []
                for k0, kp in K1_TILES:
                    t = xp.tile([kp, NCH], F32R, name="xt", tag=f"x_{k0}")
                    nc.sync.dma_start(out=t[:], in_=xT[k0 : k0 + kp, n0 : n0 + NCH])
                    xts.append(t)
                return xts

            w1t = [[], []]  # [m][k] -> [kp, 128]
            for k0, kp in K1_TILES:
                t = wp.tile([kp, 128], F32R, name=f"w1_0_{k0}")
                nc.sync.dma_start(out=t[:], in_=w1[k0 : k0 + kp, 0:128])
                w1t[0].append(t)
            x_pre = load_x_chunk(0)
            for k0, kp in K1_TILES:
                t = wp.tile([kp, 128], F32R, name=f"w1_1_{k0}")
                nc.sync.dma_start(out=t[:], in_=w1[k0 : k0 + kp, 128:256])
                w1t[1].append(t)
            w2t = []
            for k in range(2):
                t = wp.tile([128, H], F32R, name=f"w2_{k}")
                nc.sync.dma_start(out=t[:], in_=w2[k * 128 : (k + 1) * 128, :])
                w2t.append(t)
            w3t = []
            for k in range(2):
                t = wp.tile([128, NOUT], F32R, name=f"w3_{k}")
                nc.sync.dma_start(out=t[:], in_=w3[k * 128 : (k + 1) * 128, :])
                w3t.append(t)
            # Biases staged through a same-engine copy so the consuming
            # ACT/DVE op depends on its own engine (program order) instead
            # of adding a DMA-semaphore wait.
            b1t = []
            b2t = []
            for m in range(2):
                s = wp.tile([128, 1], F32, name=f"b1s_{m}")
                nc.sync.dma_start(out=s[:], in_=b1[m * 128 : (m + 1) * 128, :])
                t = wp.tile([128, 1], F32, name=f"b1_{m}")
                if m == 0:
                    nc.scalar.activation(t[:], s[:], AF.Copy)
                else:
                    nc.vector.tensor_copy(t[:], s[:])
                b1t.append(t)
                s = wp.tile([128, 1], F32, name=f"b2s_{m}")
                nc.sync.dma_start(out=s[:], in_=b2[m * 128 : (m + 1) * 128, :])
                t = wp.tile([128, 1], F32, name=f"b2_{m}")
                if m == 0:
                    nc.scalar.activation(t[:], s[:], AF.Copy)
                else:
                    nc.vector.tensor_copy(t[:], s[:])
                b2t.append(t)
            b3s = wp.tile([NOUT, 1], F32, name="b3s")
            nc.sync.dma_start(out=b3s[:], in_=b3[:, :])
            b3t = wp.tile([NOUT, 1], F32, name="b3")
            nc.vector.tensor_copy(b3t[:], b3s[:])

            # ---- batch-chunk pipeline ----------------------------------
            for ci, n0 in enumerate(range(0, B_LOC, NCH)):
                xts = x_pre if ci == 0 else load_x_chunk(n0)

                # layer 1: h1T = relu(W1eff.T @ xT + b1)
                h1 = []
                for m in range(2):
                    ps = pp1.tile([128, NCH], F32, name="ps1", tag=f"ps1_{m}")
                    for ki in range(nk1):
                        nc.tensor.matmul(
                            ps[:],
                            w1t[m][ki][:],
                            xts[ki][:],
                            start=(ki == 0),
                            stop=(ki == nk1 - 1),
                        )
                    h = hp.tile([128, NCH], F32R, name="h1", tag=f"h1_{m}")
                    if m == 0:
                        nc.scalar.activation(h[:], ps[:], AF.Relu, bias=b1t[m][:])
                    else:
                        nc.vector.tensor_scalar(
                            h[:], ps[:], b1t[m][:], 0.0, ALU.add, ALU.max
                        )
                    h1.append(h)

                # layer 2: h2T = relu(W2.T @ h1T + b2)
                h2 = []
                for m in range(2):
                    ps = pp2.tile([128, NCH], F32, name="ps2", tag=f"ps2_{m}")
                    for k in range(2):
                        nc.tensor.matmul(
                            ps[:],
                            w2t[k][:, m * 128 : (m + 1) * 128],
                            h1[k][:],
                            start=(k == 0),
                            stop=(k == 1),
                        )
                    h = hp.tile([128, NCH], F32R, name="h2", tag=f"h2_{m}")
                    if m == 0:
                        nc.scalar.activation(h[:], ps[:], AF.Relu, bias=b2t[m][:])
                    else:
                        nc.vector.tensor_scalar(
                            h[:], ps[:], b2t[m][:], 0.0, ALU.add, ALU.max
                        )
                    h2.append(h)

                # layer 3: oT = W3.T @ h2T + b3
                ps = pp3.tile([NOUT, NCH], F32, name="ps3", tag="ps3")
                for k in range(2):
                    nc.tensor.matmul(
                        ps[:],
                        w3t[k][:],
                        h2[k][:],
                        start=(k == 0),
                        stop=(k == 1),
                    )
                ob = op.tile([NOUT, NCH], F32, name="ob", tag="ob")
                nc.vector.tensor_scalar(ob[:], ps[:], b3t[:], None, ALU.add)
                # store on the ACT engine's HWDGE ring so it never queues
                # behind x loads on the SP ring
                nc.scalar.dma_start(out=outT[:, n0 : n0 + NCH], in_=ob[:])

    nc.compile()
    return nc


def _fold_conv_into_w1(conv_w: np.ndarray, W1: np.ndarray) -> np.ndarray:
    """W1eff[784, 256] such that x @ W1eff == conv_flat(x, conv_w) @ W1."""
    W1v = W1.astype(np.float64).reshape(26, 26, W1.shape[1])
    cw = conv_w.astype(np.float64)
    acc = np.zeros((28, 28, W1.shape[1]), np.float64)
    for di in range(3):
        for dj in range(3):
            acc[di : di + 26, dj : dj + 26, :] += cw[di, dj] * W1v
    return acc.reshape(KIN, W1.shape[1]).astype(np.float32)


def _run(inputs: dict, trace: bool = False, tmpdir: str | None = None):
    x = np.ascontiguousarray(np.asarray(inputs["x"], dtype=np.float32))
    w1e = _fold_conv_into_w1(np.asarray(inputs["conv_w"]), np.asarray(inputs["W1"]))
    b1 = np.ascontiguousarray(np.asarray(inputs["b1"], np.float32).reshape(H, 1))
    w2 = np.ascontiguousarray(np.asarray(inputs["W2"], np.float32))
    b2 = np.ascontiguousarray(np.asarray(inputs["b2"], np.float32).reshape(H, 1))
    w3 = np.ascontiguousarray(np.asarray(inputs["W3"], np.float32))
    b3 = np.ascontiguousarray(np.asarray(inputs["b3"], np.float32).reshape(NOUT, 1))

    nc = build_nc()
    in_maps = []
    for c in range(N_CORES):
        xs = np.ascontiguousarray(x[c * B_LOC : (c + 1) * B_LOC].T)
        in_maps.append(
            {"xT": xs, "w1": w1e, "b1": b1, "w2": w2, "b2": b2, "w3": w3, "b3": b3}
        )

    res = run_bass_kernel_spmd(
        nc, in_maps, list(range(N_CORES)), trace=trace, tmpdir=tmpdir
    )
    out = np.concatenate([r["outT"].T for r in res.results], axis=0)
    return np.ascontiguousarray(out.astype(np.float32)), res


def kernel(**inputs) -> np.ndarray:
    out, _ = _run(inputs, trace=False)
    return out


# revision 9
# speedup vs baseline: 1.0938x; 1.0839x over previous
"""Trainium2 Bass kernel for nn_DigitConvolutionalModel (3x3 conv + 3-layer MLP).

Math: out = relu(relu(conv3x3(x) @ W1 + b1) @ W2 + b2) @ W3 + b3.

The 3x3 valid conv is linear, so on host we fold it into the first FC:
  conv_flat = x @ A  with A [784, 676] (9 shifted diagonals of conv_w), so
  h1 = relu(x @ W1eff + b1)  with  W1eff = A @ W1 : [784, 256].

Sharding: pure data parallel over the batch across 8 cores (2048 rows each).
Each core runs a feature-major 3-layer MLP (activations stored transposed so
every matmul uses the weights as stored, with zero on-device transposes):
  h1T = relu(W1eff.T @ xT + b1)   [256, 2048]
  h2T = relu(W2.T   @ h1T + b2)   [256, 2048]
  oT  =      W3.T   @ h2T + b3    [10, 2048]
The host transposes each x shard on the way in and the outputs on the way out.
"""

import numpy as np

import concourse.bacc as bacc
import concourse.bass as bass
import concourse.mybir as mybir
import concourse.tile as tile
from concourse.bass_utils import run_bass_kernel_spmd

N_CORES = 8
B = 16384
B_LOC = B // N_CORES  # 2048 batch rows per core
NCH = 512  # batch chunk per matmul (fp32 PSUM bank = 512 floats)
KIN = 784  # folded input features (28*28)
H = 256
NOUT = 10

F32 = mybir.dt.float32
F32R = mybir.dt.float32r
F16 = mybir.dt.float16
AF = mybir.ActivationFunctionType
ALU = mybir.AluOpType

# K-tiles of the 784-long contraction: six of 128 plus a 16 tail
K1_TILES = [(k, min(128, KIN - k)) for k in range(0, KIN, 128)]


def build_nc() -> bass.Bass:
    nc = bacc.Bacc(
        "TRN2", target_bir_lowering=False, debug=False, num_devices=N_CORES
    )
    xT = nc.dram_tensor("xT", [KIN, B_LOC], F16, kind="ExternalInput")
    w1 = nc.dram_tensor("w1", [KIN, H], F16, kind="ExternalInput")
    b1 = nc.dram_tensor("b1", [H, 1], F32, kind="ExternalInput")
    w2 = nc.dram_tensor("w2", [H, H], F16, kind="ExternalInput")
    b2 = nc.dram_tensor("b2", [H, 1], F32, kind="ExternalInput")
    w3 = nc.dram_tensor("w3", [H, NOUT], F16, kind="ExternalInput")
    b3 = nc.dram_tensor("b3", [NOUT, 1], F32, kind="ExternalInput")
    outT = nc.dram_tensor("outT", [NOUT, B_LOC], F32, kind="ExternalOutput")

    nk1 = len(K1_TILES)

    with tile.TileContext(nc) as tc:
        with (
            tc.tile_pool(name="wgt", bufs=1) as wp,
            tc.tile_pool(name="xin", bufs=2) as xp,
            tc.tile_pool(name="act", bufs=2) as hp,
            tc.tile_pool(name="osb", bufs=2) as op,
            tc.tile_pool(name="ps1", bufs=2, space="PSUM") as pp1,
            tc.tile_pool(name="ps2", bufs=1, space="PSUM") as pp2,
            tc.tile_pool(name="ps3", bufs=2, space="PSUM") as pp3,
        ):
            # DMA issue order is consumption order (the SDMA engines
            # round-robin across everything in flight, so whatever is queued
            # first finishes first — front-load only what chunk 0 needs):
            # W1[m=0] slices + chunk-0 x, then W1[m=1], then the small stuff.
            def load_x_chunk(n0):
                xts = []
                for k0, kp in K1_TILES:
                    t = xp.tile([kp, NCH], F16, name="xt", tag=f"x_{k0}")
                    nc.sync.dma_start(out=t[:], in_=xT[k0 : k0 + kp, n0 : n0 + NCH])
                    xts.append(t)
                return xts

            w1t = [[], []]  # [m][k] -> [kp, 128]
            for k0, kp in K1_TILES:
                t = wp.tile([kp, 128], F16, name=f"w1_0_{k0}")
                nc.sync.dma_start(out=t[:], in_=w1[k0 : k0 + kp, 0:128])
                w1t[0].append(t)
            x_pre = load_x_chunk(0)
            for k0, kp in K1_TILES:
                t = wp.tile([kp, 128], F16, name=f"w1_1_{k0}")
                nc.sync.dma_start(out=t[:], in_=w1[k0 : k0 + kp, 128:256])
                w1t[1].append(t)
            w2t = []
            for k in range(2):
                t = wp.tile([128, H], F16, name=f"w2_{k}")
                nc.sync.dma_start(out=t[:], in_=w2[k * 128 : (k + 1) * 128, :])
                w2t.append(t)
            w3t = []
            for k in range(2):
                t = wp.tile([128, NOUT], F16, name=f"w3_{k}")
                nc.sync.dma_start(out=t[:], in_=w3[k * 128 : (k + 1) * 128, :])
                w3t.append(t)
            # Biases staged through a same-engine copy so the consuming
            # ACT/DVE op depends on its own engine (program order) instead
            # of adding a DMA-semaphore wait.
            b1t = []
            b2t = []
            for m in range(2):
                s = wp.tile([128, 1], F32, name=f"b1s_{m}")
                nc.sync.dma_start(out=s[:], in_=b1[m * 128 : (m + 1) * 128, :])
                t = wp.tile([128, 1], F32, name=f"b1_{m}")
                if m == 0:
                    nc.scalar.activation(t[:], s[:], AF.Copy)
                else:
                    nc.vector.tensor_copy(t[:], s[:])
                b1t.append(t)
                s = wp.tile([128, 1], F32, name=f"b2s_{m}")
                nc.sync.dma_start(out=s[:], in_=b2[m * 128 : (m + 1) * 128, :])
                t = wp.tile([128, 1], F32, name=f"b2_{m}")
                if m == 0:
                    nc.scalar.activation(t[:], s[:], AF.Copy)
                else:
                    nc.vector.tensor_copy(t[:], s[:])
                b2t.append(t)
            b3s = wp.tile([NOUT, 1], F32, name="b3s")
            nc.sync.dma_start(out=b3s[:], in_=b3[:, :])
            b3t = wp.tile([NOUT, 1], F32, name="b3")
            nc.vector.tensor_copy(b3t[:], b3s[:])

            # ---- batch-chunk pipeline ----------------------------------
            for ci, n0 in enumerate(range(0, B_LOC, NCH)):
                xts = x_pre if ci == 0 else load_x_chunk(n0)

                # layer 1: h1T = relu(W1eff.T @ xT + b1)
                h1 = []
                for m in range(2):
                    ps = pp1.tile([128, NCH], F32, name="ps1", tag=f"ps1_{m}")
                    for ki in range(nk1):
                        nc.tensor.matmul(
                            ps[:],
                            w1t[m][ki][:],
                            xts[ki][:],
                            start=(ki == 0),
                            stop=(ki == nk1 - 1),
                        )
                    h = hp.tile([128, NCH], F16, name="h1", tag=f"h1_{m}")
                    if m == 0:
                        nc.scalar.activation(h[:], ps[:], AF.Relu, bias=b1t[m][:])
                    else:
                        nc.vector.tensor_scalar(
                            h[:], ps[:], b1t[m][:], 0.0, ALU.add, ALU.max
                        )
                    h1.append(h)

                # layer 2: h2T = relu(W2.T @ h1T + b2)
                h2 = []
                for m in range(2):
                    ps = pp2.tile([128, NCH], F32, name="ps2", tag=f"ps2_{m}")
                    for k in range(2):
                        nc.tensor.matmul(
                            ps[:],
                            w2t[k][:, m * 128 : (m + 1) * 128],
                            h1[k][:],
                            start=(k == 0),
                            stop=(k == 1),
                        )
                    h = hp.tile([128, NCH], F16, name="h2", tag=f"h2_{m}")
                    if m == 0:
                        nc.scalar.activation(h[:], ps[:], AF.Relu, bias=b2t[m][:])
                    else:
                        nc.vector.tensor_scalar(
                            h[:], ps[:], b2t[m][:], 0.0, ALU.add, ALU.max
                        )
                    h2.append(h)

                # layer 3: oT = W3.T @ h2T + b3
                ps = pp3.tile([NOUT, NCH], F32, name="ps3", tag="ps3")
                for k in range(2):
                    nc.tensor.matmul(
                        ps[:],
                        w3t[k][:],
                        h2[k][:],
                        start=(k == 0),
                        stop=(k == 1),
                    )
                ob = op.tile([NOUT, NCH], F32, name="ob", tag="ob")
                nc.vector.tensor_scalar(ob[:], ps[:], b3t[:], None, ALU.add)
                # store on the ACT engine's HWDGE ring so it never queues
                # behind x loads on the SP ring
                nc.scalar.dma_start(out=outT[:, n0 : n0 + NCH], in_=ob[:])

    nc.compile()
    return nc


def _fold_conv_into_w1(conv_w: np.ndarray, W1: np.ndarray) -> np.ndarray:
    """W1eff[784, 256] such that x @ W1eff == conv_flat(x, conv_w) @ W1."""
    W1v = W1.astype(np.float64).reshape(26, 26, W1.shape[1])
    cw = conv_w.astype(np.float64)
    acc = np.zeros((28, 28, W1.shape[1]), np.float64)
    for di in range(3):
        for dj in range(3):
            acc[di : di + 26, dj : dj + 26, :] += cw[di, dj] * W1v
    return acc.reshape(KIN, W1.shape[1]).astype(np.float32)


def _run(inputs: dict, trace: bool = False, tmpdir: str | None = None):
    x = np.ascontiguousarray(np.asarray(inputs["x"], dtype=np.float32))
    w1e = _fold_conv_into_w1(np.asarray(inputs["conv_w"]), np.asarray(inputs["W1"])).astype(np.float16)
    b1 = np.ascontiguousarray(np.asarray(inputs["b1"], np.float32).reshape(H, 1))
    w2 = np.ascontiguousarray(np.asarray(inputs["W2"], np.float16))
    b2 = np.ascontiguousarray(np.asarray(inputs["b2"], np.float32).reshape(H, 1))
    w3 = np.ascontiguousarray(np.asarray(inputs["W3"], np.float16))
    b3 = np.ascontiguousarray(np.asarray(inputs["b3"], np.float32).reshape(NOUT, 1))

    nc = build_nc()
    in_maps = []
    for c in range(N_CORES):
        xs = np.ascontiguousarray(x[c * B_LOC : (c + 1) * B_LOC].T.astype(np.float16))
        in_maps.append(
            {"xT": xs, "w1": w1e, "b1": b1, "w2": w2, "b2": b2, "w3": w3, "b3": b3}
        )

    res = run_bass_kernel_spmd(
        nc, in_maps, list(range(N_CORES)), trace=trace, tmpdir=tmpdir
    )
    out = np.concatenate([r["outT"].T for r in res.results], axis=0)
    return np.ascontiguousarray(out.astype(np.float32)), res


def kernel(**inputs) -> np.ndarray:
    out, _ = _run(inputs, trace=False)
    return out


# revision 10
# speedup vs baseline: 1.4808x; 1.3538x over previous
"""Trainium2 Bass kernel for nn_DigitConvolutionalModel (3x3 conv + 3-layer MLP).

Math: out = relu(relu(conv3x3(x) @ W1 + b1) @ W2 + b2) @ W3 + b3.

The 3x3 valid conv is linear, so on host we fold it into the first FC:
  conv_flat = x @ A  with A [784, 676] (9 shifted diagonals of conv_w), so
  h1 = relu(x @ W1eff + b1)  with  W1eff = A @ W1 : [784, 256].
K is zero-padded 784 -> 896 = 7*128 so every K-tile is a full 128 partitions
and each operand loads with a single batched DMA (the SP sequencer costs
~600ns per dma_start, so few big DMAs beat many small ones).

Sharding: pure data parallel over the batch across 8 cores (2048 rows each).
Each core runs a feature-major 3-layer MLP (activations stored transposed so
every matmul uses the weights as stored, with zero on-device transposes):
  h1T = relu(W1eff.T @ xT + b1)   [256, 2048]
  h2T = relu(W2.T   @ h1T + b2)   [256, 2048]
  oT  =      W3.T   @ h2T + b3    [10, 2048]
Matmuls run in fp16 (full-rate PE streaming + FWL weight loads) with fp32
PSUM accumulation; biases and the output stay fp32. The host transposes and
fp16-casts each x shard on the way in and transposes outputs on the way out.
"""

import numpy as np

import concourse.bacc as bacc
import concourse.bass as bass
import concourse.mybir as mybir
import concourse.tile as tile
from concourse.bass_utils import run_bass_kernel_spmd

N_CORES = 8
B = 16384
B_LOC = B // N_CORES  # 2048 batch rows per core
NCH = 512  # batch chunk per matmul (fp32 PSUM bank = 512 floats)
KIN = 784  # folded input features (28*28)
KPAD = 896  # zero-padded to 7 full 128-row K-tiles
NK1 = KPAD // 128
H = 256
NOUT = 10

F32 = mybir.dt.float32
F16 = mybir.dt.float16
AF = mybir.ActivationFunctionType
ALU = mybir.AluOpType


def build_nc() -> bass.Bass:
    nc = bacc.Bacc(
        "TRN2", target_bir_lowering=False, debug=False, num_devices=N_CORES
    )
    xT = nc.dram_tensor("xT", [KPAD, B_LOC], F16, kind="ExternalInput")
    w1 = nc.dram_tensor("w1", [KPAD, H], F16, kind="ExternalInput")
    w2 = nc.dram_tensor("w2", [H, H], F16, kind="ExternalInput")
    w3 = nc.dram_tensor("w3", [H, NOUT], F16, kind="ExternalInput")
    # biases packed on host: col 0-1 = b1 (m=0,1), 2-3 = b2, 4 = b3 (10 rows)
    bias = nc.dram_tensor("bias", [128, 5], F32, kind="ExternalInput")
    outT = nc.dram_tensor("outT", [NOUT, B_LOC], F32, kind="ExternalOutput")

    with tile.TileContext(nc) as tc:
        with (
            tc.tile_pool(name="wgt", bufs=1) as wp,
            tc.tile_pool(name="xin", bufs=2) as xp,
            tc.tile_pool(name="act", bufs=2) as hp,
            tc.tile_pool(name="osb", bufs=2) as op,
            tc.tile_pool(name="ps1", bufs=2, space="PSUM") as pp1,
            tc.tile_pool(name="ps2", bufs=1, space="PSUM") as pp2,
            tc.tile_pool(name="ps3", bufs=2, space="PSUM") as pp3,
        ):
            # ---- batched loads (DMA issue order == consumption order) ----
            # W1 packed [128, 7*256]: col block k holds rows k*128..+128
            w1s = wp.tile([128, NK1 * H], F16, name="w1s")
            nc.sync.dma_start(
                out=w1s[:].rearrange("p (k c) -> p k c", c=H),
                in_=w1.rearrange("(k p) c -> p k c", p=128),
            )

            def load_x_chunk(n0):
                # one DMA: [128, 7*512], col block k = rows k*128..+128 of xT
                xc = xp.tile([128, NK1 * NCH], F16, name="xc", tag="xc")
                nc.sync.dma_start(
                    out=xc[:].rearrange("p (k n) -> p k n", n=NCH),
                    in_=xT.rearrange("(k p) n -> p k n", p=128)[:, :, n0 : n0 + NCH],
                )
                return xc

            x_pre = load_x_chunk(0)

            w2s = wp.tile([128, 2 * H], F16, name="w2s")
            nc.sync.dma_start(
                out=w2s[:].rearrange("p (k c) -> p k c", c=H),
                in_=w2.rearrange("(k p) c -> p k c", p=128),
            )
            w3s = wp.tile([128, 2 * NOUT], F16, name="w3s")
            nc.sync.dma_start(
                out=w3s[:].rearrange("p (k c) -> p k c", c=NOUT),
                in_=w3.rearrange("(k p) c -> p k c", p=128),
            )
            bs = wp.tile([128, 5], F32, name="bs")
            nc.sync.dma_start(out=bs[:], in_=bias[:, :])

            # Per-engine bias staging (consumer then depends on its own
            # engine in program order instead of an extra DMA semaphore).
            ba = wp.tile([128, 5], F32, name="ba")  # ACT's copy
            nc.scalar.activation(ba[:], bs[:], AF.Copy)
            bv = wp.tile([128, 5], F32, name="bv")  # DVE's copy
            nc.vector.tensor_copy(bv[:], bs[:])
            b1a = [ba[:, 0:1], ba[:, 1:2]]
            b2a = [ba[:, 2:3], ba[:, 3:4]]
            b1v = [bv[:, 0:1], bv[:, 1:2]]
            b2v = [bv[:, 2:3], bv[:, 3:4]]
            b3v = bv[0:NOUT, 4:5]

            # ---- batch-chunk pipeline ----------------------------------
            for ci, n0 in enumerate(range(0, B_LOC, NCH)):
                xc = x_pre if ci == 0 else load_x_chunk(n0)

                # layer 1: h1T = relu(W1eff.T @ xT + b1)
                h1 = []
                for m in range(2):
                    ps = pp1.tile([128, NCH], F32, name="ps1", tag=f"ps1_{m}")
                    for k in range(NK1):
                        nc.tensor.matmul(
                            ps[:],
                            w1s[:, k * H + m * 128 : k * H + (m + 1) * 128],
                            xc[:, k * NCH : (k + 1) * NCH],
                            start=(k == 0),
                            stop=(k == NK1 - 1),
                        )
                    h = hp.tile([128, NCH], F16, name="h1", tag=f"h1_{m}")
                    if m == 0:
                        nc.scalar.activation(h[:], ps[:], AF.Relu, bias=b1a[m])
                    else:
                        nc.vector.tensor_scalar(
                            h[:], ps[:], b1v[m], 0.0, ALU.add, ALU.max
                        )
                    h1.append(h)

                # layer 2: h2T = relu(W2.T @ h1T + b2)
                h2 = []
                for m in range(2):
                    ps = pp2.tile([128, NCH], F32, name="ps2", tag=f"ps2_{m}")
                    for k in range(2):
                        nc.tensor.matmul(
                            ps[:],
                            w2s[:, k * H + m * 128 : k * H + (m + 1) * 128],
                            h1[k][:],
                            start=(k == 0),
                            stop=(k == 1),
                        )
                    h = hp.tile([128, NCH], F16, name="h2", tag=f"h2_{m}")
                    if m == 0:
                        nc.scalar.activation(h[:], ps[:], AF.Relu, bias=b2a[m])
                    else:
                        nc.vector.tensor_scalar(
                            h[:], ps[:], b2v[m], 0.0, ALU.add, ALU.max
                        )
                    h2.append(h)

                # layer 3: oT = W3.T @ h2T + b3
                ps = pp3.tile([NOUT, NCH], F32, name="ps3", tag="ps3")
                for k in range(2):
                    nc.tensor.matmul(
                        ps[:],
                        w3s[:, k * NOUT : (k + 1) * NOUT],
                        h2[k][:],
                        start=(k == 0),
                        stop=(k == 1),
                    )
                ob = op.tile([NOUT, NCH], F32, name="ob", tag="ob")
                nc.vector.tensor_scalar(ob[:], ps[:], b3v, None, ALU.add)
                # store on the ACT engine's HWDGE ring so stores never queue
                # behind x loads on the SP ring
                nc.scalar.dma_start(out=outT[:, n0 : n0 + NCH], in_=ob[:])

    nc.compile()
    return nc


def _fold_conv_into_w1(conv_w: np.ndarray, W1: np.ndarray) -> np.ndarray:
    """W1eff[784, 256] such that x @ W1eff == conv_flat(x, conv_w) @ W1."""
    W1v = W1.astype(np.float64).reshape(26, 26, W1.shape[1])
    cw = conv_w.astype(np.float64)
    acc = np.zeros((28, 28, W1.shape[1]), np.float64)
    for di in range(3):
        for dj in range(3):
            acc[di : di + 26, dj : dj + 26, :] += cw[di, dj] * W1v
    return acc.reshape(KIN, W1.shape[1]).astype(np.float32)


def _run(inputs: dict, trace: bool = False, tmpdir: str | None = None):
    x = np.asarray(inputs["x"], dtype=np.float32)
    w1e = np.zeros((KPAD, H), np.float16)
    w1e[:KIN] = _fold_conv_into_w1(
        np.asarray(inputs["conv_w"]), np.asarray(inputs["W1"])
    ).astype(np.float16)
    w2 = np.ascontiguousarray(np.asarray(inputs["W2"], np.float16))
    w3 = np.ascontiguousarray(np.asarray(inputs["W3"], np.float16))
    bias = np.zeros((128, 5), np.float32)
    bias[:, 0:2] = np.asarray(inputs["b1"], np.float32).reshape(2, 128).T
    bias[:, 2:4] = np.asarray(inputs["b2"], np.float32).reshape(2, 128).T
    bias[:NOUT, 4] = np.asarray(inputs["b3"], np.float32)

    nc = build_nc()
    in_maps = []
    for c in range(N_CORES):
        xs = np.zeros((KPAD, B_LOC), np.float16)
        xs[:KIN] = x[c * B_LOC : (c + 1) * B_LOC].T.astype(np.float16)
        in_maps.append({"xT": xs, "w1": w1e, "w2": w2, "w3": w3, "bias": bias})

    res = run_bass_kernel_spmd(
        nc, in_maps, list(range(N_CORES)), trace=trace, tmpdir=tmpdir
    )
    out = np.concatenate([r["outT"].T for r in res.results], axis=0)
    return np.ascontiguousarray(out.astype(np.float32)), res


def kernel(**inputs) -> np.ndarray:
    out, _ = _run(inputs, trace=False)
    return out
